# revision 1
# baseline (speedup 1.0000x reference)
"""Trainium2 Bass kernel for the analog-crossbar CustomLayer.

Math (per 512x512 weight tile, per reference.py):
    cond   = (w - wmin)*s + G_MIN ; quantize to 16 levels
    g_eff  = 1/(1/cond + r_wire)          (Jeong nonlinear IV model)
    cur    = x @ g_eff ; ideal = x @ cond
    out    = ((cur - mean(cur))*coeff + mean(ideal) - offset)/s , coeff from
             per-row ranges of ideal/cur; summed over in_tiles, plus bias.

Sharding: out_tiles (columns of weight) across 8 cores; x replicated.
Each core computes a [1024, 512] slice; host concatenates.

Device mapping highlights:
  - matmuls in float32r (FP22 truncation, full PE speed at N=512)
  - ideal matmul runs against the exact integer quantization levels (fp8e4,
    exact for 0..15), reconstructed as ideal = step*ideal' + G_MIN*rowsum
  - round() via the +-1.5*2^23 trick on tensor_scalar (round-half-even,
    matches jnp.round)
  - reciprocals via Ln/Exp on ScalarE (g = q * exp(-ln(1 + q*r)))
  - per-row sums via ScalarE activation accum_out; max/min via VectorE
    reduces; in_tile accumulation via PE identity-matmul into PSUM
"""

import numpy as np
import sys

sys.path.insert(0, "/opt/trn_rl_repo")

# ---- problem constants (hardcoded; must match reference) ----
R_HRS = 1.0e6
R_LRS = 1.0e4
RP = 2.0
BITS = 4
TS = 512
G_MIN = np.float32(1.0 / R_HRS)
G_MAX = np.float32(1.0 / R_LRS)
B = 1024          # batch
IN_F = 4096       # in features
OUT_F = 4096      # out features
NCORES = 8
IT = IN_F // TS   # 8 in tiles
KC = TS // 128    # 4 k-chunks per tile
MB = B // 128     # 8 batch chunks
C_MAGIC = 12582912.0  # 1.5 * 2**23, round-to-nearest-even trick

_CACHE = {}


def _build():
    import concourse.bass as bass
    import concourse.tile as tile
    from concourse import bacc, mybir

    f32 = mybir.dt.float32
    f32r = mybir.dt.float32r
    f8 = mybir.dt.float8e4
    Alu = mybir.AluOpType
    Act = mybir.ActivationFunctionType

    nc = bacc.Bacc(None, target_bir_lowering=False, debug=False)

    xt_d = nc.dram_tensor("xt", [IN_F, B], f32, kind="ExternalInput")
    w_d = nc.dram_tensor("w", [IN_F, TS], f32, kind="ExternalInput")
    rw_d = nc.dram_tensor("rwire", [128, KC * TS], f32, kind="ExternalInput")
    scal_d = nc.dram_tensor("scal", [128, 5 * IT], f32, kind="ExternalInput")
    rsum_d = nc.dram_tensor("rsum", [MB, 128, IT], f32, kind="ExternalInput")
    biasb_d = nc.dram_tensor("biasb", [128, TS], f32, kind="ExternalInput")
    id_d = nc.dram_tensor("ident", [128, 128], f32, kind="ExternalInput")
    out_d = nc.dram_tensor("out", [B, TS], f32, kind="ExternalOutput")

    # w rows (t c p) -> per tile t: [128, c, o] chunk layout
    w_r = w_d.ap().rearrange("(t c p) o -> t p c o", t=IT, c=KC, p=128)
    # xT rows (c p) -> [128, chunk, m-col]
    xt_r = xt_d.ap().rearrange("(c p) m -> p c m", p=128)

    with tile.TileContext(nc) as tc:
        with (
            tc.tile_pool(name="const", bufs=1) as constp,
            tc.tile_pool(name="gq", bufs=1) as gqp,
            tc.tile_pool(name="wstage", bufs=1) as wstagep,
            tc.tile_pool(name="wscratch", bufs=1) as wscr,
            tc.tile_pool(name="xm", bufs=1) as xmp,
            tc.tile_pool(name="curbuf", bufs=1) as curp,
            tc.tile_pool(name="idsc", bufs=3) as idscp,
            tc.tile_pool(name="tsc", bufs=3) as tscp,
            tc.tile_pool(name="stats", bufs=2) as statp,
            tc.tile_pool(name="outsb", bufs=2) as outp,
            tc.tile_pool(name="psA", bufs=2, space=bass.MemorySpace.PSUM) as psA,
            tc.tile_pool(name="psB", bufs=2, space=bass.MemorySpace.PSUM) as psB,
            tc.tile_pool(name="psO", bufs=2, space=bass.MemorySpace.PSUM) as psO,
        ):
            # ---- constants ----
            rw_sb = constp.tile([128, KC * TS], f32)
            nc.sync.dma_start(out=rw_sb[:], in_=rw_d.ap()[:])
            scal_sb = constp.tile([128, 5 * IT], f32)
            nc.sync.dma_start(out=scal_sb[:], in_=scal_d.ap()[:])
            biasb_sb = constp.tile([128, TS], f32)
            nc.sync.dma_start(out=biasb_sb[:], in_=biasb_d.ap()[:])
            id_sb = constp.tile([128, 128], f32r)
            nc.gpsimd.dma_start(out=id_sb[:], in_=id_d.ap()[:])

            g_all = gqp.tile([128, IT * KC * TS], f32r)    # g_eff, chunk layout
            q_all = gqp.tile([128, IT * KC * TS], f32r)    # quantized conductances

            def wmin_s(it):  # broadcast per-tile scalars (slot 4)
                return scal_sb[:, 4 * IT + it:4 * IT + it + 1]

            def a_s(it):
                return scal_sb[:, IT + it:IT + it + 1]

            def stepinvs_s(it):
                return scal_sb[:, 2 * IT + it:2 * IT + it + 1]

            def stepinvs512_s(it):
                return scal_sb[:, 3 * IT + it:3 * IT + it + 1]

            STEP = float(np.float32(G_MAX - G_MIN) / np.float32(2 ** BITS - 1))

            # ================= Phase W: weight tile -> g_eff, qlev ==========
            for it in range(IT):
                wt3 = wstagep.tile([128, KC, TS], f32, tag="wt")
                nc.sync.dma_start(out=wt3[:], in_=w_r[it])
                wt = wt3[:].rearrange("p c o -> p (c o)")

                sa = wscr.tile([128, KC * TS], f32, tag="wsA")
                qsl = q_all[:, it * KC * TS:(it + 1) * KC * TS]
                gsl = g_all[:, it * KC * TS:(it + 1) * KC * TS]

                # t1 = (w - wmin) * (s/step)
                nc.vector.tensor_scalar(out=sa[:], in0=wt,
                                        scalar1=wmin_s(it), scalar2=a_s(it),
                                        op0=Alu.subtract, op1=Alu.mult)
                # rlev = round(t1)  (round-half-even via magic constant)
                nc.vector.tensor_scalar(out=sa[:], in0=sa[:],
                                        scalar1=C_MAGIC, scalar2=-C_MAGIC,
                                        op0=Alu.add, op1=Alu.add)
                # q = rlev*step + G_MIN  (persistent)
                nc.vector.tensor_scalar(out=qsl, in0=sa[:],
                                        scalar1=STEP, scalar2=float(G_MIN),
                                        op0=Alu.mult, op1=Alu.add)
                # qr = q * r_wire
                nc.vector.tensor_tensor(out=sa[:], in0=qsl, in1=rw_sb[:],
                                        op=Alu.mult)
                # ln(1 + qr), then exp(-ln) on ScalarE
                nc.scalar.activation(sa[:], sa[:], Act.Ln, bias=1.0, scale=1.0)
                nc.scalar.activation(sa[:], sa[:], Act.Exp, bias=0.0, scale=-1.0)
                # g = q * exp(-ln(1+qr)) = 1/(1/q + r)
                nc.vector.tensor_tensor(out=gsl, in0=qsl, in1=sa[:], op=Alu.mult)

            # ================= Phase X: batch chunks ========================
            for m in range(MB):
                xm = xmp.tile([128, IT * KC, 128], f32r, tag="xm")
                nc.gpsimd.dma_start(out=xm[:], in_=xt_r[:, :, m * 128:(m + 1) * 128])
                rs = statp.tile([128, IT], f32, tag="rs")
                nc.sync.dma_start(out=rs[:], in_=rsum_d.ap()[m])

                curbuf = curp.tile([128, IT * TS], f32, tag="cur")
                cmaxb = statp.tile([128, IT], f32, tag="cmax")
                cminb = statp.tile([128, IT], f32, tag="cmin")
                imaxb = statp.tile([128, IT], f32, tag="imax")
                iminb = statp.tile([128, IT], f32, tag="imin")
                csumb = statp.tile([128, IT], f32, tag="csum")
                isumb = statp.tile([128, IT], f32, tag="isum")

                for it in range(IT):
                    cur_ps = psA.tile([128, TS], f32, tag="cur_ps")
                    id_ps = psB.tile([128, TS], f32, tag="id_ps")
                    for k in range(KC):
                        lhs = xm[:, it * KC + k, :]
                        nc.tensor.matmul(
                            cur_ps[:], lhs,
                            g_all[:, (it * KC + k) * TS:(it * KC + k + 1) * TS],
                            start=(k == 0), stop=(k == KC - 1))
                    for k in range(KC):
                        lhs = xm[:, it * KC + k, :]
                        nc.tensor.matmul(
                            id_ps[:], lhs,
                            q_all[:, (it * KC + k) * TS:(it * KC + k + 1) * TS],
                            start=(k == 0), stop=(k == KC - 1))

                    # drain + row sums on ScalarE
                    cslice = curbuf[:, it * TS:(it + 1) * TS]
                    nc.scalar.activation(cslice, cur_ps[:], Act.Identity,
                                         bias=0.0, scale=1.0,
                                         accum_out=csumb[:, it:it + 1])
                    idsc = idscp.tile([128, TS], f32, tag="idsc")
                    nc.scalar.activation(idsc[:], id_ps[:], Act.Identity,
                                         bias=0.0, scale=1.0,
                                         accum_out=isumb[:, it:it + 1])
                    # per-row max/min on VectorE
                    nc.vector.tensor_reduce(cmaxb[:, it:it + 1], cslice,
                                            axis=mybir.AxisListType.X, op=Alu.max)
                    nc.vector.tensor_reduce(cminb[:, it:it + 1], cslice,
                                            axis=mybir.AxisListType.X, op=Alu.min)
                    nc.vector.tensor_reduce(imaxb[:, it:it + 1], idsc[:],
                                            axis=mybir.AxisListType.X, op=Alu.max)
                    nc.vector.tensor_reduce(iminb[:, it:it + 1], idsc[:],
                                            axis=mybir.AxisListType.X, op=Alu.min)

                # ---- batched per-row coefficients over [128, IT] ----
                di = statp.tile([128, IT], f32, tag="di")
                dc = statp.tile([128, IT], f32, tag="dc")
                co = statp.tile([128, IT], f32, tag="co")
                Ab = statp.tile([128, IT], f32, tag="Ab")
                t1 = statp.tile([128, IT], f32, tag="t1")
                t2 = statp.tile([128, IT], f32, tag="t2")
                t3 = statp.tile([128, IT], f32, tag="t3")
                Db = statp.tile([128, IT], f32, tag="Db")

                nc.vector.tensor_tensor(out=di[:], in0=imaxb[:], in1=iminb[:],
                                        op=Alu.subtract)
                # dc = (cmax + 1e-8) - cmin
                nc.vector.scalar_tensor_tensor(out=dc[:], in0=cmaxb[:],
                                               scalar=1e-8, in1=cminb[:],
                                               op0=Alu.add, op1=Alu.subtract)
                nc.vector.reciprocal(out=dc[:], in_=dc[:])
                nc.vector.tensor_tensor(out=co[:], in0=di[:], in1=dc[:],
                                        op=Alu.mult)
                # A = coeff0 * step/s ; scal columns broadcast per tile
                nc.vector.tensor_tensor(out=Ab[:], in0=co[:],
                                        in1=scal_sb[:, 2 * IT:3 * IT], op=Alu.mult)
                # D = isum'*step/(512 s) + rsum*wmin - csum*step/(512 s)*coeff0
                nc.vector.tensor_tensor(out=t1[:], in0=isumb[:],
                                        in1=scal_sb[:, 3 * IT:4 * IT], op=Alu.mult)
                nc.vector.tensor_tensor(out=t2[:], in0=rs[:],
                                        in1=scal_sb[:, 0:IT], op=Alu.mult)
                nc.vector.tensor_tensor(out=t3[:], in0=csumb[:],
                                        in1=scal_sb[:, 3 * IT:4 * IT], op=Alu.mult)
                nc.vector.tensor_tensor(out=t3[:], in0=t3[:], in1=co[:],
                                        op=Alu.mult)
                nc.vector.tensor_tensor(out=Db[:], in0=t1[:], in1=t2[:],
                                        op=Alu.subtract)
                nc.vector.tensor_tensor(out=Db[:], in0=Db[:], in1=t3[:],
                                        op=Alu.subtract)

                # ---- scale pass + accumulate over it via PE ----
                out_ps = psO.tile([128, TS], f32, tag="out_ps")
                for it in range(IT):
                    tsc = tscp.tile([128, TS], f32r, tag="tsc")
                    nc.scalar.activation(tsc[:], curbuf[:, it * TS:(it + 1) * TS],
                                         Act.Identity,
                                         bias=Db[:, it:it + 1],
                                         scale=Ab[:, it:it + 1])
                    nc.tensor.matmul(out_ps[:], id_sb[:],
                                     tsc[:],
                                     start=(it == 0), stop=(it == IT - 1))

                osb = outp.tile([128, TS], f32, tag="osb")
                nc.vector.tensor_tensor(out=osb[:], in0=out_ps[:],
                                        in1=biasb_sb[:], op=Alu.add)
                nc.sync.dma_start(out=out_d.ap()[m * 128:(m + 1) * 128, :],
                                  in_=osb[:])

    nc.compile()
    return nc


def _host_prep(x, weight, bias):
    """Build per-core input maps. All scalar math in float32."""
    x = np.ascontiguousarray(x, dtype=np.float32)
    weight = np.ascontiguousarray(weight, dtype=np.float32)
    bias = np.ascontiguousarray(bias, dtype=np.float32)

    xt = np.ascontiguousarray(x.T)                      # [4096, 1024]
    rsum = x.reshape(B, IT, TS).sum(axis=2, dtype=np.float32)  # [1024, 8]
    rsum_r = np.ascontiguousarray(
        rsum.reshape(MB, 128, IT), dtype=np.float32)    # [m, p, it]

    wr = weight.reshape(IT, TS, NCORES, TS)
    wmin = wr.min(axis=(1, 3))                          # [it, d] f32
    wmax = wr.max(axis=(1, 3))
    gr = np.float32(G_MAX) - np.float32(G_MIN)
    s = (gr / (wmax - wmin + np.float32(1e-12))).astype(np.float32)
    step = np.float32(gr / np.float32(2 ** BITS - 1))
    a = (s / step).astype(np.float32)
    invs = (np.float32(1.0) / s).astype(np.float32)
    invs512 = (invs / np.float32(512.0)).astype(np.float32)
    goff = (np.float32(G_MIN) * invs - wmin).astype(np.float32)

    # r_wire in chunk layout [128, 4*512]
    i_glob = (np.arange(KC)[:, None, None] * 128 +
              np.arange(128)[None, :, None]).astype(np.float32)
    j = np.arange(TS, dtype=np.float32)[None, None, :]
    rw = (np.float32(RP) * ((np.float32(TS) - i_glob) + (j + np.float32(1.0))))
    rw = np.ascontiguousarray(
        rw.transpose(1, 0, 2).reshape(128, KC * TS), dtype=np.float32)

    ident = np.eye(128, dtype=np.float32)

    in_maps = []
    for d in range(NCORES):
        scal = np.empty((128, 5 * IT), dtype=np.float32)
        scal[:, 0:IT] = goff[:, d][None, :]
        scal[:, IT:2 * IT] = a[:, d][None, :]
        scal[:, 2 * IT:3 * IT] = invs[:, d][None, :]
        scal[:, 3 * IT:4 * IT] = invs512[:, d][None, :]
        scal[:, 4 * IT:5 * IT] = wmin[:, d][None, :]
        in_maps.append({
            "xt": xt,
            "w": np.ascontiguousarray(weight[:, d * TS:(d + 1) * TS]),
            "rwire": rw,
            "scal": scal,
            "rsum": rsum_r,
            "biasb": np.ascontiguousarray(
                np.broadcast_to(bias[d * TS:(d + 1) * TS], (128, TS))),
            "ident": ident,
        })
    return in_maps


def get_nc():
    if "nc" not in _CACHE:
        _CACHE["nc"] = _build()
    return _CACHE["nc"]


def kernel(x, weight, bias):
    from concourse.bass_utils import run_bass_kernel_spmd

    nc = get_nc()
    in_maps = _host_prep(x, weight, bias)
    res = run_bass_kernel_spmd(nc, in_maps, core_ids=list(range(NCORES)))
    out = np.empty((B, OUT_F), dtype=np.float32)
    for d in range(NCORES):
        out[:, d * TS:(d + 1) * TS] = res.results[d]["out"]
    return out

